# revision 6
# baseline (speedup 1.0000x reference)
"""Trainium2 Bass kernel for nn_H_DYNA_42348377538865 (scatter_memory GRU + memory attention).

Self-contained: shards node dim N=512 across 8 NeuronCores (64 nodes/core),
runs a fully-unrolled 24-step recurrence per core, gathers on host.

Layout: feature-on-partitions, (node, batch) on free dim (col = n_local*32 + b,
NB=2048 cols/core). v3 design notes (vs the v1 scatter kernel):
  - single activation table: sigmoid computed as 0.5+0.5*tanh(x/2) so the
    Scalar engine only ever uses {Exp, Tanh, Copy/Identity} -> zero
    LoadActFuncSet swaps in steady state (each swap is 1283ns).
  - biases folded into matmuls via a const-1.0 row (row 64) of the state
    tiles; encode inputs x_0..x_11 pre-staged as rows 65:77 with per-step
    weight rows selecting them (no per-step x DMA).
  - decode x-feedback (x_t = y_{t-1}) folded algebraically into the decode
    gate weights (rank-1 update W + ow@Wx) plus one extra matmul for the
    candidate path; y itself is never computed on device - decode h states
    are DMA'd out and y = ow@h + bo is applied on host.
  - GRU update via fused scalar_tensor_tensor ops (all operands partition-
    aligned at 0): rh = (s_r+1)*h with the *0.5 folded into Wc; dl = hc-h on
    gpsimd; h += 0.5*(s_z+1)*dl as two fused DVE ops.
  - rolling q-cache as v1: 12 slots in 3x[128,NB] tiles; slot j pairs with
    memory slice s=(j-t)%12 via 12 precomputed rotation stacks.
"""
import numpy as np
import sys

for _p in ("/opt/trn_rl_repo",):
    if _p not in sys.path:
        sys.path.append(_p)

import concourse.bass as bass
import concourse.bacc as bacc
import concourse.mybir as mybir
import concourse.tile as tile
from concourse import bass_utils

B, T, HORIZON, N = 32, 12, 12, 512
IN, OUT, H, P = 1, 1, 64, 32
S, ML, MG, DE = 12, 64, 32, 10
NCORES = 8
NL = N // NCORES        # 64
NB = NL * B             # 2048
NSTEP = T + HORIZON     # 24
CH = 4                  # column chunks
CW = NB // CH           # 512

F32 = mybir.dt.float32
BF16 = mybir.dt.bfloat16
AF = mybir.ActivationFunctionType
ALU = mybir.AluOpType


def build_nc():
    nc = bacc.Bacc("TRN2", target_bir_lowering=False, debug=False)
    d = {}
    d["xsrc"] = nc.dram_tensor("xsrc", [T, NB], BF16, kind="ExternalInput")
    d["memstack"] = nc.dram_tensor("memstack", [128, S * 3 * 96], BF16, kind="ExternalInput")
    d["nsw"] = nc.dram_tensor("nsw", [64, NL * 64], BF16, kind="ExternalInput")
    d["fmean"] = nc.dram_tensor("fmean", [96, 64], BF16, kind="ExternalInput")
    d["fsum"] = nc.dram_tensor("fsum", [96, 64], BF16, kind="ExternalInput")
    d["zwa"] = nc.dram_tensor("zwa", [77, T * 64], BF16, kind="ExternalInput")
    d["rwa"] = nc.dram_tensor("rwa", [77, T * 64], BF16, kind="ExternalInput")
    d["cwa"] = nc.dram_tensor("cwa", [77, T * 64], BF16, kind="ExternalInput")
    d["zwd"] = nc.dram_tensor("zwd", [65, 64], BF16, kind="ExternalInput")
    d["rwd"] = nc.dram_tensor("rwd", [65, 64], BF16, kind="ExternalInput")
    d["cwd"] = nc.dram_tensor("cwd", [65, 64], BF16, kind="ExternalInput")
    d["wxd"] = nc.dram_tensor("wxd", [65, 64], BF16, kind="ExternalInput")
    d["qw2"] = nc.dram_tensor("qw2", [65, 32], BF16, kind="ExternalInput")
    d["bq4"] = nc.dram_tensor("bq4", [128, 1], F32, kind="ExternalInput")
    hs_d = nc.dram_tensor("hsave", [HORIZON * 64, NB], BF16, kind="ExternalOutput")

    with tile.TileContext(nc) as tc:
        with (
            tc.tile_pool(name="consts", bufs=1) as cp,
            tc.tile_pool(name="sp", bufs=3) as sp,
            tc.tile_pool(name="pp_lg", bufs=1, space="PSUM") as pp_lg,
            tc.tile_pool(name="pp_fu", bufs=1, space="PSUM") as pp_fu,
            tc.tile_pool(name="pp_su", bufs=1, space="PSUM") as pp_su,
            tc.tile_pool(name="pp_zp", bufs=1, space="PSUM") as pp_zp,
            tc.tile_pool(name="pp_rp", bufs=1, space="PSUM") as pp_rp,
            tc.tile_pool(name="pp_acc", bufs=2, space="PSUM") as pp_acc,
            tc.tile_pool(name="pp_qp", bufs=1, space="PSUM") as pp_qp,
        ):
            msk = cp.tile([128, S * 3 * 96], BF16)
            nc.sync.dma_start(msk[:], d["memstack"].ap())
            nsw = cp.tile([64, NL * 64], BF16)
            nc.sync.dma_start(nsw[:], d["nsw"].ap())
            fmean = cp.tile([96, 64], BF16)
            nc.sync.dma_start(fmean[:], d["fmean"].ap())
            fsum = cp.tile([96, 64], BF16)
            nc.sync.dma_start(fsum[:], d["fsum"].ap())
            zwa = cp.tile([77, T * 64], BF16)
            nc.sync.dma_start(zwa[:], d["zwa"].ap())
            rwa = cp.tile([77, T * 64], BF16)
            nc.sync.dma_start(rwa[:], d["rwa"].ap())
            cwa = cp.tile([77, T * 64], BF16)
            nc.sync.dma_start(cwa[:], d["cwa"].ap())
            zwd = cp.tile([65, 64], BF16)
            nc.sync.dma_start(zwd[:], d["zwd"].ap())
            rwd = cp.tile([65, 64], BF16)
            nc.sync.dma_start(rwd[:], d["rwd"].ap())
            cwd = cp.tile([65, 64], BF16)
            nc.sync.dma_start(cwd[:], d["cwd"].ap())
            wxd = cp.tile([65, 64], BF16)
            nc.sync.dma_start(wxd[:], d["wxd"].ap())
            qw2 = cp.tile([65, 32], BF16)
            nc.sync.dma_start(qw2[:], d["qw2"].ap())
            bq4 = cp.tile([128, 1], F32)
            nc.sync.dma_start(bq4[:], d["bq4"].ap())

            # state: rows 0:64 h (bf16), row 64 const 1.0, rows 65:77 x_0..x_11
            hx = cp.tile([77, NB], BF16)
            nc.vector.memset(hx[0:64, :], 0.0)
            nc.vector.memset(hx[64:65, :], 1.0)
            nc.sync.dma_start(hx[65:77, :], d["xsrc"].ap())
            rhx = cp.tile([77, NB], BF16)
            nc.vector.memset(rhx[0:64, :], 0.0)
            nc.vector.memset(rhx[64:65, :], 1.0)
            nc.sync.dma_start(rhx[65:77, :], d["xsrc"].ap())

            # rolling q-cache: zero-h query is q = bq
            qb = []
            for g in range(3):
                q = cp.tile([128, NB], BF16, name=f"qb{g}")
                nc.vector.memset(q[:], 0.0)
                nc.scalar.activation(q[:], q[:], AF.Identity, bias=bq4[:, 0:1])
                qb.append(q)

            for t in range(NSTEP):
                r = t % S
                j = t % S
                g_w, row_w = j // 4, (j % 4) * 32
                dec = t >= T
                if t < T:
                    ws = slice(t * 64, (t + 1) * 64)
                    zw_ap, rw_ap, cw_ap = zwa[:, ws], rwa[:, ws], cwa[:, ws]
                    nrow = 77
                elif t == T:  # decode step 0 reuses x = xs[T-1]
                    ws = slice((T - 1) * 64, T * 64)
                    zw_ap, rw_ap, cw_ap = zwa[:, ws], rwa[:, ws], cwa[:, ws]
                    nrow = 77
                else:
                    zw_ap, rw_ap, cw_ap = zwd[:], rwd[:], cwd[:]
                    nrow = 65
                for c in range(CH):
                    cs = slice(c * CW, (c + 1) * CW)
                    # attention logits vs both memory banks (96 rows)
                    lg = pp_lg.tile([96, CW], F32, tag="lg")
                    for g in range(3):
                        off = (r * 3 + g) * 96
                        nc.tensor.matmul(
                            lg[:], msk[:, off : off + 96], qb[g][:, cs],
                            start=(g == 0), stop=(g == 2),
                        )
                    ex = sp.tile([96, CW], BF16, tag="ex")
                    nc.scalar.activation(ex[:], lg[:], AF.Exp)
                    # fused mean context (raw) + replicated softmax denominators
                    fu = pp_fu.tile([64, CW], F32, tag="fu")
                    nc.tensor.matmul(fu[:], fmean[:], ex[:], start=True, stop=True)
                    su = pp_su.tile([64, CW], F32, tag="su")
                    nc.tensor.matmul(su[:], fsum[:], ex[:], start=True, stop=True)
                    rt = sp.tile([64, CW], F32, tag="rt")
                    nc.vector.reciprocal_approx_fast(rt[:], su[:])
                    fn = sp.tile([64, CW], BF16, tag="fn")
                    nc.vector.tensor_mul(fn[:], fu[:], rt[:])
                    # gates via tanh (same act table as exp):
                    # sigmoid(v) = .5 + .5*tanh(.5*v)
                    zp = pp_zp.tile([64, CW], F32, tag="zp")
                    nc.tensor.matmul(zp[:], zw_ap, hx[0:nrow, cs], start=True, stop=True)
                    rp = pp_rp.tile([64, CW], F32, tag="rp")
                    nc.tensor.matmul(rp[:], rw_ap, hx[0:nrow, cs], start=True, stop=True)
                    sz = sp.tile([64, CW], BF16, tag="sz")
                    nc.scalar.activation(sz[:], zp[:], AF.Tanh, scale=0.5)
                    sr = sp.tile([64, CW], BF16, tag="sr")
                    nc.scalar.activation(sr[:], rp[:], AF.Tanh, scale=0.5)
                    # rhx h-rows = (s_r+1)*h = 2*r*h  (cw weights pre-halved)
                    nc.vector.scalar_tensor_tensor(
                        rhx[0:64, cs], sr[:], 1.0, hx[0:64, cs],
                        op0=ALU.add, op1=ALU.mult,
                    )
                    # candidate accumulation
                    acc = pp_acc.tile([64, CW], F32, tag="acc")
                    first = True
                    if t > T:
                        nc.tensor.matmul(
                            acc[:], wxd[:], hx[0:65, cs],
                            start=True, stop=False, skip_group_check=True,
                        )
                        first = False
                    nc.tensor.matmul(
                        acc[:], cw_ap, rhx[0:nrow, cs],
                        start=first, stop=False, skip_group_check=True,
                    )
                    for k in range(16):
                        n = c * 16 + k
                        nc.tensor.matmul(
                            acc[:, k * 32 : (k + 1) * 32],
                            nsw[:, n * 64 : (n + 1) * 64],
                            fn[:, k * 32 : (k + 1) * 32],
                            start=False, stop=(k == 15), skip_group_check=True,
                        )
                    hcs = sp.tile([64, CW], BF16, tag="hcs")
                    nc.scalar.activation(hcs[:], acc[:], AF.Tanh)
                    # h += z*(hc-h) with z = .5*(s_z+1)
                    dl = sp.tile([64, CW], BF16, tag="dl")
                    nc.gpsimd.tensor_sub(dl[:], hcs[:], hx[0:64, cs])
                    dl2 = sp.tile([64, CW], BF16, tag="dl2")
                    nc.vector.scalar_tensor_tensor(
                        dl2[:], sz[:], 1.0, dl[:], op0=ALU.add, op1=ALU.mult,
                    )
                    nc.vector.scalar_tensor_tensor(
                        hx[0:64, cs], dl2[:], 0.5, hx[0:64, cs],
                        op0=ALU.mult, op1=ALU.add,
                    )
                    # q(h_t) lands directly on the target qbuf slot partitions
                    qp = pp_qp.tile([128, CW], F32, tag="qp")
                    nc.tensor.matmul(
                        qp[row_w : row_w + 32, :], qw2[:], hx[0:65, cs],
                        start=True, stop=True, tile_position=(0, row_w),
                    )
                    nc.scalar.activation(
                        qb[g_w][row_w : row_w + 32, cs], qp[row_w : row_w + 32, :],
                        AF.Copy,
                    )
                    if dec:
                        dstep = t - T
                        nc.sync.dma_start(
                            hs_d.ap()[dstep * 64 : (dstep + 1) * 64, cs],
                            hx[0:64, cs],
                        )
    nc.compile()
    return nc


def precompute(inp):
    lm = np.asarray(inp["local_mem"], np.float32)
    gm = np.asarray(inp["global_mem"], np.float32)
    Wq = np.asarray(inp["Wq"], np.float32)
    bq = np.asarray(inp["bq"], np.float32)
    node_emb = np.asarray(inp["node_emb"], np.float32)
    wp = np.asarray(inp["weight_pool"], np.float32)
    Wz = np.asarray(inp["Wz"], np.float32)
    bz = np.asarray(inp["bz"], np.float32)
    Wr = np.asarray(inp["Wr"], np.float32)
    br = np.asarray(inp["br"], np.float32)
    Wc = np.asarray(inp["Wc"], np.float32)
    bc = np.asarray(inp["bc"], np.float32)
    Wo = np.asarray(inp["Wo"], np.float32)
    bo = np.asarray(inp["bo"], np.float32)
    ow = Wo[:, 0]         # [H]
    bo0 = float(bo[0])

    c = {}
    c["nsw_full"] = np.einsum("nd,dfh->nfh", node_emb, wp).astype(np.float32)
    # memory rotation stacks (q-slot j at step r pairs with mem slice (j-r)%S)
    memsl = np.concatenate([lm.transpose(2, 0, 1), gm.transpose(2, 0, 1)], axis=1)  # [P,96,S]
    ms = np.zeros((128, S, 3, 96), np.float32)
    for r in range(S):
        for g in range(3):
            for i in range(4):
                s = (4 * g + i - r) % S
                ms[32 * i : 32 * (i + 1), r, g, :] = memsl[:, :, s]
    c["memstack"] = ms.reshape(128, S * 3 * 96)
    lmean, gmean = lm.mean(axis=1), gm.mean(axis=1)
    fme = np.zeros((96, 64), np.float32)
    fme[:ML, :P] = lmean
    fme[ML:, P : 2 * P] = gmean
    c["fmean"] = fme
    fsu = np.zeros((96, 64), np.float32)
    fsu[:ML, :P] = 1.0
    fsu[ML:, P : 2 * P] = 1.0
    c["fsum"] = fsu

    def enc_w(W, b, half=False):
        # [77, T, 64]: rows 0:64 Wh (maybe halved), row 64 bias, row 65+t Wx
        w = np.zeros((77, T, 64), np.float32)
        w[:H, :, :] = (0.5 if half else 1.0) * W[1:][:, None, :]
        w[H, :, :] = b[None, :]
        for t in range(T):
            w[H + 1 + t, t, :] = W[0]
        return w.reshape(77, T * 64)

    c["zwa"] = enc_w(Wz, bz)
    c["rwa"] = enc_w(Wr, br)
    c["cwa"] = enc_w(Wc, bc, half=True)
    # decode weights: x = y_prev folded as rank-1 update (x = ow@h_prev + bo)
    zwd = np.zeros((65, 64), np.float32)
    zwd[:H] = Wz[1:] + np.outer(ow, Wz[0])
    zwd[H] = bz + bo0 * Wz[0]
    c["zwd"] = zwd
    rwd = np.zeros((65, 64), np.float32)
    rwd[:H] = Wr[1:] + np.outer(ow, Wr[0])
    rwd[H] = br + bo0 * Wr[0]
    c["rwd"] = rwd
    cwd = np.zeros((65, 64), np.float32)
    cwd[:H] = 0.5 * Wc[1:]
    cwd[H] = bc
    c["cwd"] = cwd
    wxd = np.zeros((65, 64), np.float32)
    wxd[:H] = np.outer(ow, Wc[0])
    wxd[H] = bo0 * Wc[0]
    c["wxd"] = wxd
    qw2 = np.zeros((65, 32), np.float32)
    qw2[:H] = Wq
    qw2[H] = bq
    c["qw2"] = qw2
    c["bq4"] = np.tile(bq, 4).reshape(128, 1)
    c["ow"] = ow
    c["bo0"] = bo0
    return c


def _bf16(a):
    import ml_dtypes
    return np.ascontiguousarray(a).astype(ml_dtypes.bfloat16)


def make_in_maps(inp):
    c = precompute(inp)
    src = np.asarray(inp["source"], np.float32)
    shared = {
        "memstack": _bf16(c["memstack"]), "fmean": _bf16(c["fmean"]),
        "fsum": _bf16(c["fsum"]), "zwa": _bf16(c["zwa"]), "rwa": _bf16(c["rwa"]),
        "cwa": _bf16(c["cwa"]), "zwd": _bf16(c["zwd"]), "rwd": _bf16(c["rwd"]),
        "cwd": _bf16(c["cwd"]), "wxd": _bf16(c["wxd"]), "qw2": _bf16(c["qw2"]),
        "bq4": c["bq4"],
    }
    in_maps = []
    for core in range(NCORES):
        nodes = slice(core * NL, (core + 1) * NL)
        xs = _bf16(src[:, :, nodes, 0].transpose(1, 2, 0).reshape(T, NB))
        nswc = _bf16(c["nsw_full"][nodes].transpose(1, 0, 2).reshape(64, NL * 64))
        in_maps.append(dict(shared, xsrc=xs, nsw=nswc))
    return in_maps


def assemble(results, ow, bo0):
    out = np.zeros((B, HORIZON, N, OUT), np.float32)
    for core in range(NCORES):
        nodes = slice(core * NL, (core + 1) * NL)
        hs = np.asarray(results[core]["hsave"], np.float32)  # [HORIZON*64, NB]
        hs = hs.reshape(HORIZON, 64, NL, B)
        ys = np.einsum("k,dknb->dnb", ow, hs) + bo0          # [HORIZON, NL, B]
        out[:, :, nodes, 0] = ys.transpose(2, 0, 1)
    return out


_NC_CACHE = {}


def kernel(**inputs):
    if "nc" not in _NC_CACHE:
        _NC_CACHE["nc"] = build_nc()
    nc = _NC_CACHE["nc"]
    c_ow = np.asarray(inputs["Wo"], np.float32)[:, 0]
    c_bo = float(np.asarray(inputs["bo"], np.float32)[0])
    in_maps = make_in_maps(inputs)
    res = bass_utils.run_bass_kernel_spmd(nc, in_maps, core_ids=list(range(NCORES)))
    return assemble(res.results, c_ow, c_bo)
